# revision 13
# baseline (speedup 1.0000x reference)
"""HardGateMOE Trainium2 kernel: expert x F-slice parallel across 8 cores.

Sharding: experts are split into 2 groups of 4 (paired by routed-token
count so per-slot padded widths are minimal); each group is processed by
4 cores, each core owning an F/4 slice of its 4 experts' W1/W2. A core
runs, for each of its 4 expert slots s (token columns off_s..off_s+Ws):
  hT_s = gelu(W1[e_s][:, q*FQ:(q+1)*FQ].T @ xgT_s + b1_s)   # [FQ, Ws]
  yT_s = W2[e_s][q*FQ:(q+1)*FQ, :].T @ hT_s + b2_s          # [H, Ws] partial
The host sums the 4 F-quarter partials per expert, then applies the gate
weights (gate GEMM + token-axis softmax + combine on host). Compared to
1 expert/core this removes the max-vs-mean token imbalance (padded cols
1944 vs 2016) while keeping weight DMA volume identical.

GEMMs run in bf16 (fp8 DoubleRow is 2x/matmul but the hi/lo split needed
for the 2e-2 tolerance costs 3 matmuls = net 1.5x slower).

Schedule notes (driven by the TimelineSim cost model):
- One shared 8-buf PSUM pool spans all fc1/fc2 chains so accumulators
  only wait for their own bank's last reader (no pool-close barriers).
- Startup DMAs are ordered xg_k01, w1_k01, xg_k23, ... (2 k-slices per
  piece balances the serial HWDGE 625ns/DMA against the serial
  ~360ns/slice transfer). Biases ride the Pool/SWDGE path which does
  not contend for HWDGE. No PE warmup: the p-state ramp matures on
  wall clock before the first operands can possibly arrive.
- w2 tiles ride the Act queue but are released via tile_wait_until only
  after the fc1 feed's critical window.
- fc2 per slot staggers its last TAILF chunks per h-tile so output
  acts/DMAs drain while the next slot's fc1 runs.
"""

import ml_dtypes
import numpy as np

import concourse.tile as tile
from concourse import bacc, mybir
from concourse.bass_utils import run_bass_kernel_spmd

N, H, F, E = 2048, 1024, 4096, 8
NCORES = 8
P = 128
KH = H // P               # 8  k-chunks for fc1 (contract over H)
S = 4                     # expert slots per core
FQ = F // 4               # F-slice per core
FTQ = FQ // P             # 8  f-tiles (fc1) = k-chunks (fc2) per slot
HT = H // P               # 8  h-tiles of yT
GW = 512                  # w1 DMA group width (cols)
GF = GW // P              # 4 f-tiles per w1 group
FT = S * FTQ              # 32 (s, ftile) pairs -> b1 layout
TAILF = (5, 5, 5, 5)      # fc2 tail chunks run per-h for stagger, per slot
STARTUP = (2, 2, 2, 2)    # k-slice piece sizes for the slot-0 startup feed
OUT_Q = "alt"             # output DMA queue policy: alt | sync | scalar
NWARM = 14                # warmup matmuls: anchor the PE p-state busy-clock
                          # early so the first real matmul runs at full clock
W2_T0 = 0.0095            # slot-0 w2 DMA release (ms)
W2_SLOT = 0.0252          # per-slot w2 release offset (~ slot span)

BF16 = ml_dtypes.bfloat16

_compiled = {}
_last_nc = None


def _build(widths, repeats: int = 1):
    """Build + compile the SPMD program for slot widths (tuple of 4)."""
    widths = list(widths)
    CT = sum(widths)
    offs = [sum(widths[:s]) for s in range(S)]
    nc = bacc.Bacc("TRN2", target_bir_lowering=False, debug=False,
                   num_devices=NCORES)
    f32 = mybir.dt.float32
    bf16 = mybir.dt.bfloat16
    GELU = mybir.ActivationFunctionType.Gelu_apprx_tanh
    IDENT = mybir.ActivationFunctionType.Identity

    xg = nc.dram_tensor("xg", [P, KH, CT], bf16, kind="ExternalInput").ap()
    # w1[s*2+gg] = W1[e_s][:, q*FQ + gg*GW : ...] as [P(k-rows), KH, GW]
    w1 = nc.dram_tensor("w1", [S * 2, P, KH, GW], bf16,
                        kind="ExternalInput").ap()
    b1 = nc.dram_tensor("b1", [P, FT], f32, kind="ExternalInput").ap()
    # w2[s*FTQ+f] = W2[e_s][q*FQ + f*128 : ..., :] as [P(k-rows), H]
    w2 = nc.dram_tensor("w2", [S * FTQ, P, H], bf16,
                        kind="ExternalInput").ap()
    b2 = nc.dram_tensor("b2", [P, S * HT], f32, kind="ExternalInput").ap()
    y = nc.dram_tensor("y", [HT, P, CT], bf16, kind="ExternalOutput").ap()

    with tile.TileContext(nc) as tc:
      for _rep in range(repeats):
        with (
            tc.tile_pool(name="singles", bufs=1) as singles,
            tc.tile_pool(name="w1p", bufs=3) as w1p,
            tc.tile_pool(name="w2p", bufs=10) as w2p,
            tc.tile_pool(name="outp", bufs=8) as outp,
            tc.tile_pool(name="hTp", bufs=2) as hTp,
            tc.tile_pool(name="ps", bufs=8, space="PSUM") as ps,
        ):
            # PE warmup on a memset tile: its only job is to anchor the
            # p-state busy-clock near t=0 -- a cold first matmul after a
            # >2us idle resets the ramp and runs at half clock for 3us.
            warm = singles.tile([P, P], bf16)
            nc.vector.memset(warm, 0.0)
            for i in range(NWARM):
                wps = ps.tile([P, P], f32, tag="ps", name=f"wps{i}")
                nc.tensor.matmul(out=wps, lhsT=warm, rhs=warm,
                                 start=True, stop=True)

            xg_s = singles.tile([P, KH, CT], bf16)
            w1_g0 = w1p.tile([P, KH, GW], bf16, name="w1g0", tag="w1")
            # Startup feed for slot 0's first group, strict SP order.
            W0 = widths[0]
            k = 0
            for step in STARTUP:
                nc.sync.dma_start(out=xg_s[:, k:k + step, 0:W0],
                                  in_=xg[:, k:k + step, 0:W0])
                nc.sync.dma_start(out=w1_g0[:, k:k + step, :],
                                  in_=w1[0, :, k:k + step, :])
                k += step
            assert k == KH
            # Biases ride the Pool/SWDGE path: no HWDGE contention.
            b1_s = singles.tile([P, FT], f32)
            b2_s = singles.tile([P, S * HT], f32)
            nc.gpsimd.dma_start(out=b1_s, in_=b1)
            nc.gpsimd.dma_start(out=b2_s, in_=b2)

            hT = [hTp.tile([P, FTQ, max(widths)], bf16, tag="hT",
                           name=f"hT{i}") for i in range(2)]

            def fc1_group(s, gg, w1_t):
                W, off = widths[s], offs[s]
                gi = s * 2 + gg
                if w1_t is None:
                    w1_t = w1p.tile([P, KH, GW], bf16, name=f"w1g{gi}",
                                    tag="w1")
                    # group 1 in finer pieces: its consumption window
                    # starts before the transfer queue drains, so finer
                    # completion-sems hide the 900ns posting lag
                    step = 2 if gi == 1 else 4
                    for k in range(0, KH, step):
                        nc.sync.dma_start(out=w1_t[:, k:k + step, :],
                                          in_=w1[gi, :, k:k + step, :])
                # k-major emission: all 4 f-chains advance together so
                # consumption matches the k-slice DMA arrival order.
                pst = [ps.tile([P, W], f32, tag="ps", name=f"ps{gi}_{fl}")
                       for fl in range(GF)]
                for k in range(KH):
                    for fl in range(GF):
                        nc.tensor.matmul(
                            out=pst[fl],
                            lhsT=w1_t[:, k, fl * P:(fl + 1) * P],
                            rhs=xg_s[:, k, off:off + W],
                            start=(k == 0), stop=(k == KH - 1))
                ht = hT[s % 2]
                for fl in range(GF):
                    ft = gg * GF + fl
                    nc.scalar.activation(
                        out=ht[:, ft, 0:W], in_=pst[fl], func=GELU,
                        bias=b1_s[:, s * FTQ + ft:s * FTQ + ft + 1])

            def fc2_slot(s, w2_t):
                W, off = widths[s], offs[s]
                tailf = TAILF[s]
                ht = hT[s % 2]
                ps_y = [ps.tile([P, W], f32, tag="ps", name=f"ps_y{s}_{h}")
                        for h in range(HT)]
                for f in range(FTQ - tailf):
                    for h in range(HT):
                        nc.tensor.matmul(
                            out=ps_y[h],
                            lhsT=w2_t[f][:, h * P:(h + 1) * P],
                            rhs=ht[:, f, 0:W],
                            start=(f == 0), stop=False)
                for h in range(HT):
                    for f in range(FTQ - tailf, FTQ):
                        nc.tensor.matmul(
                            out=ps_y[h],
                            lhsT=w2_t[f][:, h * P:(h + 1) * P],
                            rhs=ht[:, f, 0:W],
                            start=False, stop=(f == FTQ - 1))
                    o_t = outp.tile([P, W], bf16, tag="y", name=f"o{s}_{h}")
                    nc.scalar.activation(
                        out=o_t, in_=ps_y[h], func=IDENT,
                        bias=b2_s[:, s * HT + h:s * HT + h + 1])
                    oq = (nc.sync if OUT_Q == "sync" else
                          nc.scalar if OUT_Q == "scalar" else
                          (nc.sync if h % 2 == 0 else nc.scalar))
                    oq.dma_start(out=y[h][:, off:off + W], in_=o_t)

            for s in range(S):
                if s > 0:
                    # slot-s tokens, emitted on SP after slot s-1's w1 so
                    # the serial transfer queue serves weights first
                    off, W = offs[s], widths[s]
                    for k in range(0, KH, 2):
                        nc.sync.dma_start(out=xg_s[:, k:k + 2, off:off + W],
                                          in_=xg[:, k:k + 2, off:off + W])
                # w2 tiles ride the Pool/SWDGE path: no HWDGE use, and
                # no Act-sequencer contention with the gelus. Released
                # per-slot; the Pool sequencer's ~1.1us/DMA paces within
                # a slot.
                w2_t = []
                for f in range(FTQ):
                    t = w2p.tile([P, H], bf16, tag="w2", name=f"w2t{s}_{f}")
                    with tc.tile_wait_until(W2_T0 + s * W2_SLOT):
                        nc.gpsimd.dma_start(out=t, in_=w2[s * FTQ + f])
                    w2_t.append(t)
                fc1_group(s, 0, w1_g0 if s == 0 else None)
                fc1_group(s, 1, None)
                fc2_slot(s, w2_t)

    nc.compile()
    return nc


def kernel(**inputs) -> np.ndarray:
    global _last_nc
    x = np.ascontiguousarray(np.asarray(inputs["x"], dtype=np.float32))
    mapping = np.asarray(inputs["mapping"]).astype(np.int64)
    Wg = np.asarray(inputs["Wg"], dtype=np.float32)
    W1 = np.asarray(inputs["W1"], dtype=np.float32)
    b1 = np.asarray(inputs["b1"], dtype=np.float32)
    W2 = np.asarray(inputs["W2"], dtype=np.float32)
    b2 = np.asarray(inputs["b2"], dtype=np.float32)

    n, h = x.shape
    assert (n, h) == (N, H)

    # Host-side dispatch: unique tokens per expert (a token routed to the
    # same expert by both slots contributes once, with summed gate weight).
    token_lists = []
    for e in range(E):
        tl = np.nonzero((mapping == e).any(axis=1))[0]
        token_lists.append(tl)
    counts = [len(tl) for tl in token_lists]

    # Pair experts by count rank: slot s of group A takes rank 2s, group B
    # rank 2s+1; slot width = the larger (group A's) count, padded to 8.
    order = sorted(range(E), key=lambda e: -counts[e])
    groupA = [order[2 * s] for s in range(S)]
    groupB = [order[2 * s + 1] for s in range(S)]
    widths = tuple(counts[groupA[s]] for s in range(S))
    CT = sum(widths)
    offs = [sum(widths[:s]) for s in range(S)]

    if widths not in _compiled:
        _compiled[widths] = _build(widths)
    nc = _compiled[widths]
    _last_nc = nc

    def pack_group(experts):
        xgT = np.zeros((H, CT), dtype=BF16)
        b2g = np.zeros((S * HT, P), dtype=np.float32)
        for s, e in enumerate(experts):
            tl = token_lists[e]
            xgT[:, offs[s]:offs[s] + len(tl)] = x[tl].T.astype(BF16)
            b2g[s * HT:(s + 1) * HT] = b2[e].reshape(HT, P)
        xgc = np.ascontiguousarray(
            xgT.reshape(KH, P, CT).transpose(1, 0, 2))
        return xgc, np.ascontiguousarray(b2g.T)

    xgA, b2A = pack_group(groupA)
    xgB, b2B = pack_group(groupB)
    b2zero = np.zeros((P, S * HT), dtype=np.float32)

    in_maps = []
    for core in range(NCORES):
        experts = groupA if core < 4 else groupB
        q = core % 4
        w1c = np.empty((S * 2, P, KH, GW), dtype=BF16)
        w2c = np.empty((S * FTQ, P, H), dtype=BF16)
        b1c = np.empty((FT, P), dtype=np.float32)
        for s, e in enumerate(experts):
            wsl = W1[e][:, q * FQ:(q + 1) * FQ]      # [H, FQ]
            # [2, P, KH, GW]: piece[gg, r, k, c] = wsl[k*128+r, gg*512+c]
            w1c[s * 2:(s + 1) * 2] = (
                wsl.reshape(KH, P, 2, GW).transpose(2, 1, 0, 3).astype(BF16))
            w2sl = W2[e][q * FQ:(q + 1) * FQ, :]     # [FQ, H]
            w2c[s * FTQ:(s + 1) * FTQ] = (
                w2sl.reshape(FTQ, P, H).astype(BF16))
            b1c[s * FTQ:(s + 1) * FTQ] = (
                b1[e][q * FQ:(q + 1) * FQ].reshape(FTQ, P))
        in_maps.append({
            "xg": xgA if core < 4 else xgB,
            "w1": w1c,
            "b1": np.ascontiguousarray(b1c.T),
            "w2": w2c,
            # the b2 bias must land once per expert, not once per
            # F-quarter partial; only quarter 0 carries it.
            "b2": (b2A if core < 4 else b2B) if q == 0 else b2zero,
        })

    res = run_bass_kernel_spmd(nc, in_maps, list(range(NCORES)))

    # Host: gate GEMM + token-axis softmax, per-(token,k) weights, combine.
    zf = x @ Wg.T                                     # [N, E]
    zf -= zf.max(axis=0, keepdims=True)
    ez = np.exp(zf)
    logits = ez / ez.sum(axis=0, keepdims=True)
    w = np.take_along_axis(logits, mapping, axis=1)
    w = w / w.sum(axis=1, keepdims=True)

    out = np.zeros((N, H), dtype=np.float32)
    for gi, experts in enumerate((groupA, groupB)):
        # sum the 4 F-quarter partials of this group's cores
        ysum = np.zeros((HT, P, CT), dtype=np.float32)
        for q in range(4):
            ysum += np.asarray(
                res.results[gi * 4 + q]["y"]).astype(np.float32)
        yT = ysum.reshape(H, CT)
        for s, e in enumerate(experts):
            tl = token_lists[e]
            cw = (w[tl, 0] * (mapping[tl, 0] == e)
                  + w[tl, 1] * (mapping[tl, 1] == e)).astype(np.float32)
            out[tl] += cw[:, None] * yT[:, offs[s]:offs[s] + len(tl)].T
    return out


# revision 21
# speedup vs baseline: 1.0106x; 1.0106x over previous
"""HardGateMOE Trainium2 kernel: expert x F-slice parallel across 8 cores.

Sharding: experts are split into 2 groups of 4 (paired by routed-token
count so per-slot padded widths are minimal); each group is processed by
4 cores, each core owning an F/4 slice of its 4 experts' W1/W2. A core
runs, for each of its 4 expert slots s (token columns off_s..off_s+Ws):
  hT_s = gelu(W1[e_s][:, q*FQ:(q+1)*FQ].T @ xgT_s + b1_s)   # [FQ, Ws]
  yT_s = W2[e_s][q*FQ:(q+1)*FQ, :].T @ hT_s + b2_s          # [H, Ws] partial
The host sums the 4 F-quarter partials per expert, then applies the gate
weights (gate GEMM + token-axis softmax + combine on host). Compared to
1 expert/core this removes the max-vs-mean token imbalance (padded cols
1944 vs 2016) while keeping weight DMA volume identical.

GEMMs run in bf16 (fp8 DoubleRow is 2x/matmul but the hi/lo split needed
for the 2e-2 tolerance costs 3 matmuls = net 1.5x slower).

Schedule notes (driven by the TimelineSim cost model):
- One shared 8-buf PSUM pool spans all fc1/fc2 chains so accumulators
  only wait for their own bank's last reader (no pool-close barriers).
- Startup DMAs are ordered xg_k01, w1_k01, xg_k23, ... (2 k-slices per
  piece balances the serial HWDGE 625ns/DMA against the serial
  ~360ns/slice transfer). Biases ride the Pool/SWDGE path which does
  not contend for HWDGE. No PE warmup: the p-state ramp matures on
  wall clock before the first operands can possibly arrive.
- w2 tiles ride the Act queue but are released via tile_wait_until only
  after the fc1 feed's critical window.
- fc2 per slot staggers its last TAILF chunks per h-tile so output
  acts/DMAs drain while the next slot's fc1 runs.
"""

import ml_dtypes
import numpy as np

import concourse.tile as tile
from concourse import bacc, mybir
from concourse.bass_utils import run_bass_kernel_spmd

N, H, F, E = 2048, 1024, 4096, 8
NCORES = 8
P = 128
KH = H // P               # 8  k-chunks for fc1 (contract over H)
S = 4                     # expert slots per core
FQ = F // 4               # F-slice per core
FTQ = FQ // P             # 8  f-tiles (fc1) = k-chunks (fc2) per slot
HT = H // P               # 8  h-tiles of yT
GW = 512                  # w1 DMA group width (cols)
GF = GW // P              # 4 f-tiles per w1 group
FT = S * FTQ              # 32 (s, ftile) pairs -> b1 layout
TAILF = (5, 5, 5, 6)      # fc2 tail chunks run per-h for stagger, per slot
STARTUP = (2, 2, 2, 2)    # k-slice piece sizes for the slot-0 startup feed
G0WIDE = 640              # slot-0 first fc1 group width (cols); rest of FQ
                          # forms the second group
OUT_Q = "sync"            # output DMA queue policy: alt | sync | scalar
B1B2_T = None             # optional release gate (ms) for bias DMAs
G1STEP = 2                # w1 group-1 DMA piece size (k-slices)
NWARM = 14                # warmup matmuls: anchor the PE p-state busy-clock
                          # early so the first real matmul runs at full clock
W2_T0 = 0.0095            # slot-0 w2 DMA release (ms)
W2_SLOT = 0.0252          # per-slot w2 release offset (~ slot span)

BF16 = ml_dtypes.bfloat16

_compiled = {}
_last_nc = None


def _build(widths, repeats: int = 1):
    """Build + compile the SPMD program for slot widths (tuple of 4)."""
    widths = list(widths)
    CT = sum(widths)
    offs = [sum(widths[:s]) for s in range(S)]
    nc = bacc.Bacc("TRN2", target_bir_lowering=False, debug=False,
                   num_devices=NCORES)
    f32 = mybir.dt.float32
    bf16 = mybir.dt.bfloat16
    GELU = mybir.ActivationFunctionType.Gelu_apprx_tanh
    IDENT = mybir.ActivationFunctionType.Identity

    xg = nc.dram_tensor("xg", [P, KH, CT], bf16, kind="ExternalInput").ap()
    # w1[s] = W1[e_s][:, q*FQ:(q+1)*FQ] as [P(k-rows), KH, FQ]
    w1 = nc.dram_tensor("w1", [S, P, KH, FQ], bf16,
                        kind="ExternalInput").ap()
    b1 = nc.dram_tensor("b1", [P, FT], f32, kind="ExternalInput").ap()
    # w2[s*FTQ+f] = W2[e_s][q*FQ + f*128 : ..., :] as [P(k-rows), H]
    w2 = nc.dram_tensor("w2", [S * FTQ, P, H], bf16,
                        kind="ExternalInput").ap()
    b2 = nc.dram_tensor("b2", [P, S * HT], f32, kind="ExternalInput").ap()
    y = nc.dram_tensor("y", [HT, P, CT], bf16, kind="ExternalOutput").ap()

    with tile.TileContext(nc) as tc:
      for _rep in range(repeats):
        with (
            tc.tile_pool(name="singles", bufs=1) as singles,
            tc.tile_pool(name="w1p", bufs=3) as w1p,
            tc.tile_pool(name="w2p", bufs=10) as w2p,
            tc.tile_pool(name="outp", bufs=8) as outp,
            tc.tile_pool(name="hTp", bufs=2) as hTp,
            tc.tile_pool(name="ps", bufs=8, space="PSUM") as ps,
        ):
            # PE warmup on a memset tile: its only job is to anchor the
            # p-state busy-clock near t=0 -- a cold first matmul after a
            # >2us idle resets the ramp and runs at half clock for 3us.
            warm = singles.tile([P, P], bf16)
            nc.vector.memset(warm, 0.0)
            for i in range(NWARM):
                wps = ps.tile([P, P], f32, tag="ps", name=f"wps{i}")
                nc.tensor.matmul(out=wps, lhsT=warm, rhs=warm,
                                 start=True, stop=True)

            xg_s = singles.tile([P, KH, CT], bf16)
            w1_g0 = w1p.tile([P, KH, G0WIDE], bf16, name="w1g0", tag="w1")
            # Startup feed: the binding rate is per-DMA issue cost
            # (~0.8us: SEQ hold + shared HWDGE 625ns), so the feed is
            # split across two queues -- xg on SP, slot-0 w1 on Act --
            # leaving only the HWDGE serialization shared.
            W0 = widths[0]
            for k in range(0, KH, 2):
                nc.sync.dma_start(out=xg_s[:, k:k + 2, 0:W0],
                                  in_=xg[:, k:k + 2, 0:W0])
                nc.sync.dma_start(out=w1_g0[:, k:k + 2, :],
                                  in_=w1[0, :, k:k + 2, 0:G0WIDE])
            # Biases ride the Pool/SWDGE path: no HWDGE contention.
            b1_s = singles.tile([P, FT], f32)
            b2_s = singles.tile([P, S * HT], f32)
            if B1B2_T is None:
                nc.gpsimd.dma_start(out=b1_s, in_=b1)
                nc.gpsimd.dma_start(out=b2_s, in_=b2)
            else:
                with tc.tile_wait_until(B1B2_T):
                    nc.gpsimd.dma_start(out=b1_s, in_=b1)
                    nc.gpsimd.dma_start(out=b2_s, in_=b2)

            hT = [hTp.tile([P, FTQ, max(widths)], bf16, tag="hT",
                           name=f"hT{i}") for i in range(2)]

            def fc1_group(s, c0, c1, ft0, w1_t, step=4, q=None):
                """fc1 chains for slot s, W1 columns [c0, c1)."""
                W, off = widths[s], offs[s]
                ntile = (c1 - c0) // P
                if w1_t is None:
                    w1_t = w1p.tile([P, KH, c1 - c0], bf16,
                                    name=f"w1g{s}_{c0}", tag="w1",
                                    padded_shape=[P, KH, max(G0WIDE, GW)])
                    for k in range(0, KH, step):
                        (q or nc.sync).dma_start(
                            out=w1_t[:, k:k + step, :],
                            in_=w1[s, :, k:k + step, c0:c1])
                # k-major emission: all f-chains advance together so
                # consumption matches the k-slice DMA arrival order.
                pst = [ps.tile([P, W], f32, tag="ps", name=f"ps{s}_{c0}_{fl}")
                       for fl in range(ntile)]
                for k in range(KH):
                    for fl in range(ntile):
                        nc.tensor.matmul(
                            out=pst[fl],
                            lhsT=w1_t[:, k, fl * P:(fl + 1) * P],
                            rhs=xg_s[:, k, off:off + W],
                            start=(k == 0), stop=(k == KH - 1))
                ht = hT[s % 2]
                for fl in range(ntile):
                    ft = ft0 + fl
                    nc.scalar.activation(
                        out=ht[:, ft, 0:W], in_=pst[fl], func=GELU,
                        bias=b1_s[:, s * FTQ + ft:s * FTQ + ft + 1])

            def fc2_slot(s, w2_t):
                W, off = widths[s], offs[s]
                tailf = TAILF[s]
                ht = hT[s % 2]
                ps_y = [ps.tile([P, W], f32, tag="ps", name=f"ps_y{s}_{h}")
                        for h in range(HT)]
                for f in range(FTQ - tailf):
                    for h in range(HT):
                        nc.tensor.matmul(
                            out=ps_y[h],
                            lhsT=w2_t[f][:, h * P:(h + 1) * P],
                            rhs=ht[:, f, 0:W],
                            start=(f == 0), stop=False)
                for h in range(HT):
                    for f in range(FTQ - tailf, FTQ):
                        nc.tensor.matmul(
                            out=ps_y[h],
                            lhsT=w2_t[f][:, h * P:(h + 1) * P],
                            rhs=ht[:, f, 0:W],
                            start=False, stop=(f == FTQ - 1))
                    o_t = outp.tile([P, W], bf16, tag="y", name=f"o{s}_{h}")
                    nc.scalar.activation(
                        out=o_t, in_=ps_y[h], func=IDENT,
                        bias=b2_s[:, s * HT + h:s * HT + h + 1])
                    oq = (nc.sync if OUT_Q == "sync" else
                          nc.scalar if OUT_Q == "scalar" else
                          (nc.sync if h % 2 == 0 else nc.scalar))
                    oq.dma_start(out=y[h][:, off:off + W], in_=o_t)

            for s in range(S):
                if s > 0:
                    # slot-s tokens, emitted on SP after slot s-1's w1 so
                    # the serial transfer queue serves weights first
                    off, W = offs[s], widths[s]
                    for k in range(0, KH, 2):
                        nc.sync.dma_start(out=xg_s[:, k:k + 2, off:off + W],
                                          in_=xg[:, k:k + 2, off:off + W])
                # w2 tiles ride the Pool/SWDGE path: no HWDGE use, and
                # no Act-sequencer contention with the gelus. Released
                # per-slot; the Pool sequencer's ~1.1us/DMA paces within
                # a slot.
                w2_t = []
                for f in range(FTQ):
                    t = w2p.tile([P, H], bf16, tag="w2", name=f"w2t{s}_{f}")
                    with tc.tile_wait_until(W2_T0 + s * W2_SLOT):
                        nc.gpsimd.dma_start(out=t, in_=w2[s * FTQ + f])
                    w2_t.append(t)
                if s == 0:
                    fc1_group(0, 0, G0WIDE, 0, w1_g0)
                    fc1_group(0, G0WIDE, FQ, G0WIDE // P, None,
                              step=G1STEP)
                else:
                    fc1_group(s, 0, GW, 0, None)
                    fc1_group(s, GW, FQ, GW // P, None)
                fc2_slot(s, w2_t)

    nc.compile()
    return nc


def kernel(**inputs) -> np.ndarray:
    global _last_nc
    x = np.ascontiguousarray(np.asarray(inputs["x"], dtype=np.float32))
    mapping = np.asarray(inputs["mapping"]).astype(np.int64)
    Wg = np.asarray(inputs["Wg"], dtype=np.float32)
    W1 = np.asarray(inputs["W1"], dtype=np.float32)
    b1 = np.asarray(inputs["b1"], dtype=np.float32)
    W2 = np.asarray(inputs["W2"], dtype=np.float32)
    b2 = np.asarray(inputs["b2"], dtype=np.float32)

    n, h = x.shape
    assert (n, h) == (N, H)

    # Host-side dispatch: unique tokens per expert (a token routed to the
    # same expert by both slots contributes once, with summed gate weight).
    token_lists = []
    for e in range(E):
        tl = np.nonzero((mapping == e).any(axis=1))[0]
        token_lists.append(tl)
    counts = [len(tl) for tl in token_lists]

    # Pair experts by count rank: slot s of group A takes rank 2s, group B
    # rank 2s+1; slot width = the larger (group A's) count, padded to 8.
    order = sorted(range(E), key=lambda e: -counts[e])
    groupA = [order[2 * s] for s in range(S)]
    groupB = [order[2 * s + 1] for s in range(S)]
    widths = tuple(counts[groupA[s]] for s in range(S))
    CT = sum(widths)
    offs = [sum(widths[:s]) for s in range(S)]

    if widths not in _compiled:
        _compiled[widths] = _build(widths)
    nc = _compiled[widths]
    _last_nc = nc

    def pack_group(experts):
        xgT = np.zeros((H, CT), dtype=BF16)
        b2g = np.zeros((S * HT, P), dtype=np.float32)
        for s, e in enumerate(experts):
            tl = token_lists[e]
            xgT[:, offs[s]:offs[s] + len(tl)] = x[tl].T.astype(BF16)
            b2g[s * HT:(s + 1) * HT] = b2[e].reshape(HT, P)
        xgc = np.ascontiguousarray(
            xgT.reshape(KH, P, CT).transpose(1, 0, 2))
        return xgc, np.ascontiguousarray(b2g.T)

    xgA, b2A = pack_group(groupA)
    xgB, b2B = pack_group(groupB)
    b2zero = np.zeros((P, S * HT), dtype=np.float32)

    in_maps = []
    for core in range(NCORES):
        experts = groupA if core < 4 else groupB
        q = core % 4
        w1c = np.empty((S, P, KH, FQ), dtype=BF16)
        w2c = np.empty((S * FTQ, P, H), dtype=BF16)
        b1c = np.empty((FT, P), dtype=np.float32)
        for s, e in enumerate(experts):
            wsl = W1[e][:, q * FQ:(q + 1) * FQ]      # [H, FQ]
            # [P, KH, FQ]: w1c[s, r, k, c] = wsl[k*128+r, c]
            w1c[s] = wsl.reshape(KH, P, FQ).transpose(1, 0, 2).astype(BF16)
            w2sl = W2[e][q * FQ:(q + 1) * FQ, :]     # [FQ, H]
            w2c[s * FTQ:(s + 1) * FTQ] = (
                w2sl.reshape(FTQ, P, H).astype(BF16))
            b1c[s * FTQ:(s + 1) * FTQ] = (
                b1[e][q * FQ:(q + 1) * FQ].reshape(FTQ, P))
        in_maps.append({
            "xg": xgA if core < 4 else xgB,
            "w1": w1c,
            "b1": np.ascontiguousarray(b1c.T),
            "w2": w2c,
            # the b2 bias must land once per expert, not once per
            # F-quarter partial; only quarter 0 carries it.
            "b2": (b2A if core < 4 else b2B) if q == 0 else b2zero,
        })

    res = run_bass_kernel_spmd(nc, in_maps, list(range(NCORES)))

    # Host: gate GEMM + token-axis softmax, per-(token,k) weights, combine.
    zf = x @ Wg.T                                     # [N, E]
    zf -= zf.max(axis=0, keepdims=True)
    ez = np.exp(zf)
    logits = ez / ez.sum(axis=0, keepdims=True)
    w = np.take_along_axis(logits, mapping, axis=1)
    w = w / w.sum(axis=1, keepdims=True)

    out = np.zeros((N, H), dtype=np.float32)
    for gi, experts in enumerate((groupA, groupB)):
        # sum the 4 F-quarter partials of this group's cores
        ysum = np.zeros((HT, P, CT), dtype=np.float32)
        for q in range(4):
            ysum += np.asarray(
                res.results[gi * 4 + q]["y"]).astype(np.float32)
        yT = ysum.reshape(H, CT)
        for s, e in enumerate(experts):
            tl = token_lists[e]
            cw = (w[tl, 0] * (mapping[tl, 0] == e)
                  + w[tl, 1] * (mapping[tl, 1] == e)).astype(np.float32)
            out[tl] += cw[:, None] * yT[:, offs[s]:offs[s] + len(tl)].T
    return out
